# revision 91
# baseline (speedup 1.0000x reference)
"""Causal multi-head self-attention on 8 Trainium2 NeuronCores.

B=2, N=2048, D=1024, H=16 heads of d=64. Head-parallel sharding: core c
owns heads 2c, 2c+1. Each core reads the full (transposed, bf16) X and its
128-column slice of Wq/Wk/Wv (and 128-row slice of Wo), computes
Q^T/K^T/V for its 2 heads, runs causal flash-style attention entirely in
"transposed" layout (zero on-device transposes), applies its Wo slice, and
writes a full-shape partial output. The host sums the 8 partials + bo.

Per-core dataflow:
  X^T (host-transposed, bf16)  --DMA-->  SBUF, 8 k-tiles [128, 4096]
  Q^T/K^T: projected bf16, evacuated to fp8 e4m3 in a [128, 2, BN]
    "DoubleRow" layout (subtile 1 zeroed once) with 1/sqrt(dk) split
    evenly between Q and K on the host.
  V = X Wv_c, stored as 32 blocks [V_h0 | 1 | V_h1 | 1] (ones columns
    compute the sum-exp row inside the AV matmul for free).
  per (batch, q-chunk of 512, k-block PAIR of 2x128, head):
    S^T pair [128, 1024] via 2 DoubleRow fp8 matmuls (2x PE throughput;
      pairing amortizes the DR<->normal weight-pipeline switch)
    ONE exp over [128, 1024] (halves ACT instruction count), mask on Pool
    2 bf16 AV matmuls accumulate AV^T + sumexp per head
    software-pipelined with one pair of lookahead; batch-1 projections
    and the output projection are pumped between steps as PE filler
  normalize: bf16 evac of av PSUM (frees the bank immediately), raw
    sumexp row broadcast via a rank-1 PE matmul (ones x row) into the
    freed bank, 1/x via the bit-trick reciprocal_approx_fast on the
    [64, nq] broadcast at base partition 0 (custom DVE ops misread
    nonzero base partitions; the exact reciprocal on a [1, 512] row is
    ~6 cycles/elem on ONE lane = 3.2us, the hidden bomb of the
    baseline), then one multiply per head.
  O_partial = (AVn^T).T @ Wo_c  -> DMA to DRAM in [128, 4, D] batches
"""

import numpy as np

B, N, D, H, DK, DV = 2, 2048, 1024, 16, 64, 64
NCORES = 8
HPC = H // NCORES  # heads per core = 2
BN = B * N  # 4096
NQ_CHUNK = 512  # query chunk (psum free dim)
NK_BLK = 128  # key block (psum partition dim)
N_JCH = N // NQ_CHUNK  # 4 q-chunks per batch
N_KBLK = N // NK_BLK  # 16 k-blocks per batch
KT_PER_D = D // 128  # 8 contraction tiles for the projections
NBLK_ALL = BN // NK_BLK  # 32 n-blocks over both batches

_STATE = {}

import os as _os

ABL = set(_os.environ.get("ABL", "").split(",")) - {""}


def _build_nc(iters=1):
    import contextlib

    import concourse.bacc as bacc
    import concourse.mybir as mybir
    import concourse.tile as tile
    from concourse.masks import make_upper_triangular

    f32 = mybir.dt.float32
    bf16 = mybir.dt.bfloat16
    fp8 = mybir.dt.float8e4
    DR = mybir.MatmulPerfMode.DoubleRow
    AF = mybir.ActivationFunctionType

    nc = bacc.Bacc("TRN2", target_bir_lowering=False, debug=False)

    xt_d = nc.dram_tensor("xt", [D, BN], bf16, kind="ExternalInput")
    wq_d = nc.dram_tensor("wq", [128, KT_PER_D, 128], bf16, kind="ExternalInput")
    wk_d = nc.dram_tensor("wk", [128, KT_PER_D, 128], bf16, kind="ExternalInput")
    wv_d = nc.dram_tensor("wv", [128, KT_PER_D, 128], bf16, kind="ExternalInput")
    wo_d = nc.dram_tensor("wo", [128, D], bf16, kind="ExternalInput")
    bq_d = nc.dram_tensor("bq", [128, 1], f32, kind="ExternalInput")
    bk_d = nc.dram_tensor("bk", [128, 1], f32, kind="ExternalInput")
    bv_d = nc.dram_tensor("bv", [128, 1], f32, kind="ExternalInput")
    out_d = nc.dram_tensor("out", [BN, D], bf16, kind="ExternalOutput")

    with tile.TileContext(nc) as tc:
        with (
            tc.tile_pool(name="const", bufs=1) as const,
            tc.tile_pool(name="xtp", bufs=KT_PER_D) as xtp,  # bufs per col-group tag
            tc.tile_pool(name="persist", bufs=1) as persist,
            tc.tile_pool(name="avn", bufs=4) as avnp,
            tc.tile_pool(name="expp", bufs=12) as expp,
            
            tc.tile_pool(name="s0p", bufs=5) as s0p,
            tc.tile_pool(name="bcp", bufs=3) as bcp,
            tc.tile_pool(name="h1tp", bufs=3) as h1tp,
            tc.tile_pool(name="osb", bufs=4) as osbp,
        ):
            # ---- constants ----
            wq_sb = const.tile([128, KT_PER_D, 128], bf16, tag="wq")
            wk_sb = const.tile([128, KT_PER_D, 128], bf16, tag="wk")
            wv_sb = const.tile([128, KT_PER_D, 128], bf16, tag="wv")
            nc.sync.dma_start(wq_sb[:], wq_d[:])
            nc.sync.dma_start(wk_sb[:], wk_d[:])
            nc.sync.dma_start(wv_sb[:], wv_d[:])
            wo_sb = const.tile([128, D], bf16, tag="wo")
            nc.sync.dma_start(wo_sb[:], wo_d[:])
            bq_sb = const.tile([128, 1], f32, tag="bq")
            bk_sb = const.tile([128, 1], f32, tag="bk")
            nc.sync.dma_start(bq_sb[:], bq_d[:])
            nc.sync.dma_start(bk_sb[:], bk_d[:])
            bv_sb = const.tile([128, 1], f32, tag="bv")
            nc.sync.dma_start(bv_sb[:], bv_d[:])
            # causal keep-mask: mask[p, f] = 1.0 iff f >= p
            mask = const.tile([128, 128], bf16, tag="mask")
            make_upper_triangular(nc, mask[:], val=1.0, diag=True)
            # ones row at partition 64 for the PE outer-product broadcast
            # (1/sumexp row -> [64, nq] tile) used by the normalize
            onesb = const.tile([128, 64], bf16, tag="onesb")
            nc.vector.memset(onesb[:], 1.0)

            # ---- X^T tiles, split by column-group so matmuls start on the
            # first 256 KB instead of after the full 8 MB ----
            NCG = 4
            CGW = BN // NCG  # 1024 cols per group
            xt_t = {}
            for cg in range(NCG):
                for k in range(KT_PER_D):
                    t = xtp.tile([128, CGW], bf16, tag=f"xt{cg}", name=f"xt{k}_{cg}")
                    nc.sync.dma_start(
                        t[:], xt_d[k * 128 : (k + 1) * 128, cg * CGW : (cg + 1) * CGW]
                    )
                    xt_t[(k, cg)] = t

            def xt_slice(k, c0, c1):
                cg = c0 // CGW
                assert c1 <= (cg + 1) * CGW
                return xt_t[(k, cg)][:, c0 - cg * CGW : c1 - cg * CGW]

            # ---- persistent activations ----
            # Q^T/K^T in fp8 as [128, 2, BN]: s=0 holds data, s=1 stays zero
            # so DoubleRow matmuls contract K*Q + 0*0 at 2 elem/cycle
            QT = persist.tile([128, 2, BN], fp8, tag="qt")
            KT = persist.tile([128, 2, BN], fp8, tag="kt")
            nc.vector.memset(QT[:, 1, :], 0.0)
            nc.vector.memset(KT[:, 1, :], 0.0)
            VT = persist.tile([128, BN], bf16, tag="vt")
            # V blocks with interleaved ones cols: [V_h0 | 1 | V_h1 | 1]
            V_sb = persist.tile([128, NBLK_ALL, 130], bf16, tag="vsb")
            vview = V_sb.rearrange("p i (g c) -> p i g c", c=65)
            nc.vector.memset(vview[:, :, :, 64:65], 1.0)

            def body():
                # PSUM budget (8 banks): pq 2 (proj + out-proj share) + s 2x2
                # ([128,1024] score-pair tiles, one per head) + av 2
                with (
                    tc.tile_pool(name="pp", bufs=2, space="PSUM") as pp,
                    tc.tile_pool(name="sp", bufs=2, space="PSUM") as sp,
                    tc.tile_pool(name="avp", bufs=2, space="PSUM") as avp,
                ):
                    op = pp
                    def emit_proj_chunk(j8, pump=False, which=("q", "k", "v"), evac_act=False):
                        """Q/K/V^T projection for one 512-col chunk; as a
                        generator it yields after each PE instruction so it
                        can be pumped as filler inside the attention loop."""
                        c0, c1 = j8 * NQ_CHUNK, (j8 + 1) * NQ_CHUNK
                        for w_sb, b_sb, dst, nm in (
                            (wq_sb, bq_sb, QT, "q"),
                            (wk_sb, bk_sb, KT, "k"),
                            (wv_sb, bv_sb, VT, "v"),
                        ):
                            if nm not in which:
                                continue
                            ps = pp.tile(
                                [128, NQ_CHUNK], f32, tag="psq", name=f"ps{nm}{j8}"
                            )
                            for k in range(KT_PER_D):
                                if "proj" in ABL and k not in (0, KT_PER_D - 1):
                                    continue
                                nc.tensor.matmul(
                                    ps[:],
                                    w_sb[:, k, :],
                                    xt_slice(k, c0, c1),
                                    start=(k == 0),
                                    stop=(k == KT_PER_D - 1),
                                )
                                if pump:
                                    yield
                            d = dst[:, c0:c1] if nm == "v" else dst[:, 0, c0:c1]
                            if evac_act:
                                # fill idle ACT at kernel start (before exps)
                                nc.scalar.activation(
                                    d, ps[:], AF.Identity, bias=b_sb[:, 0:1]
                                )
                            else:
                                nc.vector.tensor_scalar_add(d, ps[:], b_sb[:, 0:1])
                            if pump:
                                yield

                    def gen_v_chunk(j8):
                        yield from emit_proj_chunk(j8, pump=True, which=("v",))
                        emit_v_layout(0, j8)
                        yield

                    def emit_v_layout(b, q):
                        if "vlay" in ABL:
                            return
                        # one 512-col quarter: xbar transpose to contiguous
                        # scratch (strided 3D transpose output misbehaves on
                        # HW), then DVE-copy into the interleaved layout.
                        nb0 = b * N_KBLK + q * 4
                        for h in range(HPC):
                            vtmp = h1tp.tile(
                                [128, 4, 64], bf16, tag="vtmp", name=f"vtmp{b}_{q}_{h}"
                            )
                            nc.sync.dma_start_transpose(
                                vtmp[:],
                                VT[h * 64 : (h + 1) * 64, b * N + q * 512 : b * N + (q + 1) * 512],
                            )
                            nc.gpsimd.tensor_copy(
                                V_sb[:, nb0 : nb0 + 4, 65 * h : 65 * h + 64], vtmp[:]
                            )

                    def gen_b1_tail():
                        for j8 in range(N_JCH, 2 * N_JCH):
                            yield from emit_proj_chunk(j8, pump=True)
                            emit_v_layout(1, j8 - N_JCH)
                            yield

                    def gen_out(b, j, AVnj):
                        """Output projection for one normalized q-chunk."""
                        for nb in range(NQ_CHUNK // 128):
                            row0 = b * N + j * NQ_CHUNK + nb * 128
                            osb = osbp.tile(
                                [128, D], bf16, tag="osb", name=f"osb{b}_{j}_{nb}"
                            )
                            for half in range(2):
                                o_ps = op.tile(
                                    [128, 512], f32, tag="psq", name=f"o{b}_{j}_{nb}_{half}"
                                )
                                nc.tensor.matmul(
                                    o_ps[:],
                                    AVnj[:, nb * 128 : (nb + 1) * 128],
                                    wo_sb[:, half * 512 : (half + 1) * 512],
                                    start=True,
                                    stop=True,
                                )
                                nc.any.tensor_copy(
                                    osb[:, half * 512 : (half + 1) * 512], o_ps[:]
                                )
                                yield
                            nc.sync.dma_start(out_d[row0 : row0 + 128, :], osb[:])
                            yield

                    pending = []

                    def pump(n):
                        done = 0
                        while pending and done < n:
                            try:
                                next(pending[0])
                                done += 1
                            except StopIteration:
                                pending.pop(0)

                    pending.append(gen_b1_tail())

                    for b in range(B):
                        cb = b * N
                        j_order = range(N_JCH) if b == 0 else range(N_JCH - 1, -1, -1)
                        for j in j_order:
                            if b == 0:
                                # Q/K land just-in-time per chunk; V of chunk j
                                # is only read by its last 4 i-blocks, so for
                                # j>=1 it becomes front-of-queue PE filler
                                qk = ("q", "k", "v") if j == 0 else ("q", "k")
                                for _ in emit_proj_chunk(j, pump=False, which=qk,
                                                         evac_act=(j == 0)):
                                    pass
                                if j == 0:
                                    emit_v_layout(0, 0)
                                else:
                                    pending.insert(0, gen_v_chunk(j))
                                    pump(3)
                            kmax = (j + 1) * (NQ_CHUNK // NK_BLK)
                            q0 = cb + j * NQ_CHUNK
                            av_ps = [
                                avp.tile([65, NQ_CHUNK], f32, tag="av", name=f"av{b}_{j}_{h}")
                                for h in range(HPC)
                            ]

                            def block_f0(i):
                                r = i - j * (NQ_CHUNK // NK_BLK)
                                return (128 * r if r > 0 else 0), r

                            def emit_scores_pair(p):
                                """4 back-to-back DoubleRow matmuls for k-blocks
                                2p, 2p+1 of both heads: batching same-mode
                                matmuls amortizes the PE's DR<->normal weight
                                pipeline switch."""
                                tiles = []
                                for h in range(HPC):
                                    s2 = sp.tile(
                                        [128, 2 * NQ_CHUNK], f32, tag="s",
                                        name=f"s{b}_{j}_{p}_{h}",
                                    )
                                    for q in range(2):
                                        i = 2 * p + q
                                        f0, _ = block_f0(i)
                                        o = q * NQ_CHUNK
                                        if "scores" in ABL:
                                            continue
                                        nc.tensor.matmul(
                                            s2[:, o + f0 : o + NQ_CHUNK],
                                            KT[h * 64 : (h + 1) * 64, :, cb + i * 128 : cb + (i + 1) * 128],
                                            QT[h * 64 : (h + 1) * 64, :, q0 + f0 : q0 + NQ_CHUNK],
                                            start=True,
                                            stop=True,
                                            perf_mode=DR,
                                        )
                                    tiles.append(s2)
                                return tiles

                            def emit_expav_pair(p, tiles):
                                """One [128,1024] exp per head (half the ACT
                                instruction count), then 4 back-to-back bf16
                                AV matmuls."""
                                f0l, _ = block_f0(2 * p)
                                ets = []
                                for h in range(HPC):
                                    et = expp.tile(
                                        [128, 2 * NQ_CHUNK], bf16, tag="exp",
                                        name=f"e{b}_{j}_{p}_{h}",
                                    )
                                    # one exp across the pair; the dead zone
                                    # between the two f0 offsets is never read
                                    if "exp" not in ABL:
                                        nc.scalar.activation(
                                            et[:, f0l:], tiles[h][:, f0l:], AF.Exp
                                        )
                                    else:
                                        nc.vector.memset(et[:, f0l : f0l + 4], 1.0)
                                    for q in range(2):
                                        i = 2 * p + q
                                        f0, r = block_f0(i)
                                        o = q * NQ_CHUNK
                                        if r >= 0 and "mask" not in ABL:
                                            # Pool is idle and may touch SBUF
                                            nc.gpsimd.tensor_mul(
                                                et[:, o + f0 : o + f0 + 128],
                                                et[:, o + f0 : o + f0 + 128],
                                                mask[:],
                                            )
                                    ets.append(et)
                                for h in range(HPC):
                                    for q in range(2):
                                        i = 2 * p + q
                                        f0, _ = block_f0(i)
                                        o = q * NQ_CHUNK
                                        if "av" in ABL and i != 0 and i != kmax - 1:
                                            continue
                                        nc.tensor.matmul(
                                            av_ps[h][:, f0:NQ_CHUNK],
                                            V_sb[:, b * N_KBLK + i, 65 * h : 65 * (h + 1)],
                                            ets[h][:, o + f0 : o + NQ_CHUNK],
                                            start=(i == 0),
                                            stop=(i == kmax - 1),
                                            skip_group_check=True,
                                        )

                            prev = None
                            for p in range(kmax // 2):
                                cur = emit_scores_pair(p)
                                if prev is not None:
                                    pump(3)
                                    emit_expav_pair(p - 1, prev)
                                    pump(1)
                                prev = cur
                            emit_expav_pair(kmax // 2 - 1, prev)
                            # normalize this chunk straight out of PSUM
                            AVnj = avnp.tile(
                                [128, NQ_CHUNK], bf16, tag="avn", name=f"avn{b}_{j}"
                            )
                            if "norm" in ABL:
                                nc.vector.tensor_copy(AVnj[0:64, :], av_ps[0][0:64, :])
                                h1a = h1tp.tile(
                                    [64, NQ_CHUNK], bf16, tag="h1t", name=f"h1a{b}_{j}"
                                )
                                nc.vector.tensor_copy(h1a[:], av_ps[1][0:64, :])
                                nc.sync.dma_start(AVnj[64:128, :], h1a[:])
                            for h in range(HPC if "norm" not in ABL else 0):
                                # evacuate the av PSUM bank immediately (bf16
                                # copy, ~0.7us) so the next chunk's AV
                                # accumulation isn't serialized behind the
                                # multi-us recip/broadcast chain below
                                ur = s0p.tile(
                                    [65, NQ_CHUNK], bf16, tag="rc", name=f"ur{b}_{j}_{h}"
                                )
                                nc.vector.tensor_copy(ur[:], av_ps[h][:])
                                # broadcast the RAW sumexp row across 64
                                # partitions with a rank-1 PE matmul into the
                                # just-freed av bank, then take the reciprocal
                                # of the broadcast tile at base partition 0
                                # (custom DVE ops silently misread slices at
                                # nonzero base partitions)
                                bc = avp.tile(
                                    [65, NQ_CHUNK], f32, tag="av", name=f"bc{b}_{j}_{h}"
                                )
                                nc.tensor.matmul(
                                    bc[0:64, :],
                                    onesb[64:65, 0:64],
                                    ur[64:65, :],
                                    start=True,
                                    stop=True,
                                )
                                rcb = s0p.tile(
                                    [65, NQ_CHUNK], f32, tag="rcv", name=f"rcb{b}_{j}_{h}"
                                )
                                nc.vector.reciprocal_approx_fast(
                                    rcb[0:64, :], bc[0:64, :]
                                )
                                if h == 0:
                                    nc.vector.tensor_mul(
                                        AVnj[0:64, :], ur[0:64, :], rcb[0:64, :]
                                    )
                                else:
                                    h1t = h1tp.tile(
                                        [64, NQ_CHUNK], bf16, tag="h1t", name=f"h1t{b}_{j}"
                                    )
                                    nc.vector.tensor_mul(h1t[:], ur[0:64, :], rcb[0:64, :])
                                    nc.sync.dma_start(AVnj[64:128, :], h1t[:])
                            if "out" not in ABL:
                                # front of the queue: out-proj drains evenly
                                # instead of stacking up behind the batch-1
                                # projection filler and serializing at the tail
                                pending.insert(0, gen_out(b, j, AVnj))
                        if b == 0:
                            # batch-1 projections must finish before its attention
                            pump(10**9)
                    pump(10**9)

            if iters > 1:
                with tc.For_i(0, iters, 1):
                    body()
            else:
                body()

    nc.compile()
    return nc


def _prep_in_maps(X, Wq, bq, Wk, bk, Wv, bv, Wo, bo):
    import ml_dtypes

    bf16 = ml_dtypes.bfloat16

    def _pkm(w):  # [D, 128] -> [128 partition, k, 128] tile layout
        return np.ascontiguousarray(
            w.reshape(KT_PER_D, 128, 128).transpose(1, 0, 2)
        ).astype(bf16)
    scale = np.float32(1.0 / np.sqrt(np.sqrt(DK)))
    Xf = np.asarray(X, dtype=np.float32).reshape(BN, D)
    xt = np.ascontiguousarray(Xf.T).astype(bf16)
    in_maps = []
    for c in range(NCORES):
        s = slice(c * 128, (c + 1) * 128)
        in_maps.append(
            {
                "xt": xt,
                "wq": _pkm(np.asarray(Wq, np.float32)[:, s] * scale),
                "wk": _pkm(np.asarray(Wk, np.float32)[:, s] * scale),
                "wv": _pkm(np.asarray(Wv, np.float32)[:, s]),
                "wo": np.ascontiguousarray(np.asarray(Wo, np.float32)[s, :]).astype(bf16),
                "bq": np.ascontiguousarray(
                    (np.asarray(bq, np.float32)[s] * scale).reshape(128, 1)
                ),
                "bk": np.ascontiguousarray(
                    (np.asarray(bk, np.float32)[s] * scale).reshape(128, 1)
                ),
                "bv": np.ascontiguousarray(np.asarray(bv, np.float32)[s].reshape(128, 1)),
            }
        )
    return in_maps


def _get_nc(iters=1):
    key = ("nc", iters, tuple(sorted(ABL)))
    if key not in _STATE:
        _STATE[key] = _build_nc(iters)
    return _STATE[key]


def kernel(**inputs) -> np.ndarray:
    from concourse import bass_utils

    nc = _get_nc()
    in_maps = _prep_in_maps(**inputs)
    res = bass_utils.run_bass_kernel_spmd(nc, in_maps, core_ids=list(range(NCORES)))
    acc = np.zeros((BN, D), dtype=np.float32)
    for r in res.results:
        acc += np.asarray(r["out"], dtype=np.float32)
    acc += np.asarray(inputs["bo"], np.float32)[None, :]
    return acc.reshape(B, N, D)

